# revision 1
# baseline (speedup 1.0000x reference)
"""BinarizedLeNet5/CIFAR10 Trainium2 kernel (8-core data parallel).

Host side: conv1 im2col (bf16 hi+lo split => fp32-equivalent precision with
+-1 weights), weight sign-folding, BN constant folding.
Device side (per core, 256 samples in 16 chunks of 16):
  conv1: block-diag stationary [128,128] (4 row-bands x 32 ch, psum partition
         p = 4*co + j), two accumulating bf16 matmuls (hi+lo),
  sign BEFORE maxpool (monotone affine with inv>0 => exact),
  ACT: conv1 sign;  DVE: conv1 pools, conv2 pools (x via single-input reduce
  over adjacent pairs since psum allows one tensor operand), conv2 sign on ACT
  after pooling,
  conv2: 3 accumulating dy-matmuls K=96 on an SBUF shifted-im2col built by
         3 contiguous-run DMAs per chunk,
  fc1/fc2 feature-major, fc3 batch-in-psum + fused log_softmax.
"""
import sys
import numpy as np

sys.path.insert(0, "/opt/pypackages")
sys.path.insert(0, "/opt/trn_rl_repo")

import ml_dtypes

BF = ml_dtypes.bfloat16
NCORES = 8
B = 2048
BC = B // NCORES          # 256 samples per core
CH = 16                   # samples per chunk
NCHUNK = BC // CH         # 16 chunks
EPS = np.float32(1e-5)

_nc_cache = {}


def _f32(x):
    return np.asarray(x, np.float32)


def _host_prep(inputs):
    """Build all per-core device input arrays."""
    x = _f32(inputs["x"])                      # [2048,3,32,32]

    # ---- conv1 im2col, hi/lo bf16 ----
    xhi = x.astype(BF)
    xlo = (x - xhi.astype(np.float32)).astype(BF)

    def im2col(xq):
        xp = np.zeros((B, 3, 34, 34), BF)
        xp[:, :, 1:33, 1:33] = xq
        ic = np.zeros((B, 128, 8, 32), BF)
        for j in range(4):
            for dy in range(3):
                for dx in range(3):
                    t = 3 * dy + dx
                    ic[:, 32 * j + 3 * t:32 * j + 3 * t + 3] = \
                        xp[:, :, 8 * j + dy:8 * j + dy + 8, dx:dx + 32]
        # -> [core, chunk, 128, s, y, x] -> [core, chunk, 128, 4096]
        ic = ic.reshape(NCORES, NCHUNK, CH, 128, 8, 32)
        ic = np.ascontiguousarray(ic.transpose(0, 1, 3, 2, 4, 5))
        return ic.reshape(NCORES, NCHUNK, 128, CH * 256)

    ic1h = im2col(xhi)
    ic1l = im2col(xlo)

    # ---- conv1 stationary: block-diag, k = 3*(3dy+dx)+c, out p = 4*co+j ----
    w1s = np.sign(_f32(inputs["conv1_w"]))               # [32,3,3,3] (co,c,dy,dx)
    w1k = np.ascontiguousarray(w1s.transpose(2, 3, 1, 0)).reshape(27, 32)
    w1_st = np.zeros((128, 128), BF)
    for j in range(4):
        for co in range(32):
            w1_st[32 * j:32 * j + 27, 4 * co + j] = w1k[:, co].astype(BF)

    # ---- conv1 sign ACT constants (bias folded), indexed p = 4*co+j ----
    inv1 = _f32(inputs["bn1_g"]) / np.sqrt(_f32(inputs["bn1_v"]) + EPS)
    sh1c = (_f32(inputs["conv1_b"]) - _f32(inputs["bn1_m"])) * inv1 \
        + _f32(inputs["bn1_b"])
    sc1 = np.repeat(inv1, 4).reshape(128, 1).astype(np.float32)
    sh1 = np.repeat(sh1c, 4).reshape(128, 1).astype(np.float32)

    # ---- conv2 stationaries [3][96,64]: p = 32*dx + c ----
    w2s = np.sign(_f32(inputs["conv2_w"]))               # [64,32,3,3]
    w2_st = np.zeros((3, 96, 64), BF)
    for dy in range(3):
        for dx in range(3):
            w2_st[dy, 32 * dx:32 * dx + 32] = w2s[:, :, dy, dx].T.astype(BF)

    # ---- conv2 post-pool sign constants (ACT) ----
    inv2 = _f32(inputs["bn2_g"]) / np.sqrt(_f32(inputs["bn2_v"]) + EPS)
    sh2c = _f32(inputs["bn2_b"]) - _f32(inputs["bn2_m"]) * inv2
    sc2 = inv2.reshape(64, 1).astype(np.float32)
    sh2 = (_f32(inputs["conv2_b"]) * inv2 + sh2c).reshape(64, 1).astype(np.float32)

    # ---- fc1 ----
    fw1 = np.sign(_f32(inputs["fc1_w"]))                 # [512,4096]
    fc1_st = np.ascontiguousarray(
        fw1.T.reshape(32, 128, 512).astype(BF))          # [kt][k][m]
    inv3 = _f32(inputs["bn3_g"]) / np.sqrt(_f32(inputs["bn3_v"]) + EPS)
    sh3c = (_f32(inputs["fc1_b"]) - _f32(inputs["bn3_m"])) * inv3 \
        + _f32(inputs["bn3_b"])
    sc3 = np.ascontiguousarray(inv3.reshape(4, 128).T).astype(np.float32)   # [128,4]
    sh3 = np.ascontiguousarray(sh3c.reshape(4, 128).T).astype(np.float32)

    # ---- fc2 ----
    fw2 = np.sign(_f32(inputs["fc2_w"]))                 # [256,512]
    fc2_st = np.ascontiguousarray(fw2.T.reshape(4, 128, 256).astype(BF))
    inv4 = _f32(inputs["bn4_g"]) / np.sqrt(_f32(inputs["bn4_v"]) + EPS)
    sh4c = (_f32(inputs["fc2_b"]) - _f32(inputs["bn4_m"])) * inv4 \
        + _f32(inputs["bn4_b"])
    sc4 = np.ascontiguousarray(inv4.reshape(2, 128).T).astype(np.float32)   # [128,2]
    sh4 = np.ascontiguousarray(sh4c.reshape(2, 128).T).astype(np.float32)

    # ---- fc3 hi/lo ----
    w3 = _f32(inputs["fc3_w"]).T                         # [256,10]
    w3h = w3.astype(BF)
    w3l = (w3 - w3h.astype(np.float32)).astype(BF)
    w3h = np.ascontiguousarray(w3h.reshape(2, 128, 10))
    w3l = np.ascontiguousarray(w3l.reshape(2, 128, 10))
    b3bc = np.tile(_f32(inputs["fc3_b"]).reshape(1, 10), (128, 1)).astype(np.float32)

    shared = dict(w1=w1_st, w2=w2_st, fc1w=fc1_st, fc2w=fc2_st,
                  w3h=w3h, w3l=w3l, sc1=sc1, sh1=sh1, sc2=sc2, sh2=sh2,
                  sc3=sc3, sh3=sh3, sc4=sc4, sh4=sh4, b3bc=b3bc)
    in_maps = []
    for ci in range(NCORES):
        m = dict(shared)
        m["ic1h"] = np.ascontiguousarray(ic1h[ci])
        m["ic1l"] = np.ascontiguousarray(ic1l[ci])
        in_maps.append(m)
    return in_maps


def _build_module(reps=1):
    import concourse.bass as bass
    import concourse.mybir as mybir
    import concourse.tile as tile
    from concourse import bacc
    from contextlib import ExitStack

    F32 = mybir.dt.float32
    BF16 = mybir.dt.bfloat16
    AF = mybir.ActivationFunctionType
    ALU = mybir.AluOpType

    nc = bacc.Bacc("TRN2", target_bir_lowering=False, debug=False)

    # ---- DRAM tensors ----
    d_ic1h = nc.dram_tensor("ic1h", [NCHUNK, 128, CH * 256], BF16, kind="ExternalInput")
    d_ic1l = nc.dram_tensor("ic1l", [NCHUNK, 128, CH * 256], BF16, kind="ExternalInput")
    d_w1 = nc.dram_tensor("w1", [128, 128], BF16, kind="ExternalInput")
    d_w2 = nc.dram_tensor("w2", [3, 96, 64], BF16, kind="ExternalInput")
    d_fc1w = nc.dram_tensor("fc1w", [32, 128, 512], BF16, kind="ExternalInput")
    d_fc2w = nc.dram_tensor("fc2w", [4, 128, 256], BF16, kind="ExternalInput")
    d_w3h = nc.dram_tensor("w3h", [2, 128, 10], BF16, kind="ExternalInput")
    d_w3l = nc.dram_tensor("w3l", [2, 128, 10], BF16, kind="ExternalInput")
    d_sc1 = nc.dram_tensor("sc1", [128, 1], F32, kind="ExternalInput")
    d_sh1 = nc.dram_tensor("sh1", [128, 1], F32, kind="ExternalInput")
    d_sc2 = nc.dram_tensor("sc2", [64, 1], F32, kind="ExternalInput")
    d_sh2 = nc.dram_tensor("sh2", [64, 1], F32, kind="ExternalInput")
    d_sc3 = nc.dram_tensor("sc3", [128, 4], F32, kind="ExternalInput")
    d_sh3 = nc.dram_tensor("sh3", [128, 4], F32, kind="ExternalInput")
    d_sc4 = nc.dram_tensor("sc4", [128, 2], F32, kind="ExternalInput")
    d_sh4 = nc.dram_tensor("sh4", [128, 2], F32, kind="ExternalInput")
    d_b3bc = nc.dram_tensor("b3bc", [128, 10], F32, kind="ExternalInput")
    d_out = nc.dram_tensor("out", [BC, 10], F32, kind="ExternalOutput")

    NBUF = 3                       # staging ping-pong depth
    SPW = 4 * CH * 18              # sp payload: (u 4, s CH, w 18)
    SP_FREE = SPW + 8              # +slack for dx-shifted reads
    IC2_FREE = 18 * CH * 18 + 8    # (R 18, s CH, W 18)

    with tile.TileContext(nc) as tc, ExitStack() as ctx:
        const = ctx.enter_context(tc.tile_pool(name="const", bufs=1))
        icp = ctx.enter_context(tc.tile_pool(name="icp", bufs=2))
        wk = ctx.enter_context(tc.tile_pool(name="wk", bufs=3))
        pp1 = ctx.enter_context(tc.tile_pool(name="pp1", bufs=2, space="PSUM"))
        pp2 = ctx.enter_context(tc.tile_pool(name="pp2", bufs=2, space="PSUM"))

        # ---- persistent tiles ----
        w1_sb = const.tile([128, 128], BF16, tag="w1")
        w2_sb = const.tile([96, 3, 64], BF16, tag="w2")
        fc1w_sb = const.tile([128, 32, 512], BF16, tag="fc1w")
        fc2w_sb = const.tile([128, 4, 256], BF16, tag="fc2w")
        w3h_sb = const.tile([128, 2, 10], BF16, tag="w3h")
        w3l_sb = const.tile([128, 2, 10], BF16, tag="w3l")
        sc1_sb = const.tile([128, 1], F32, tag="sc1")
        sh1_sb = const.tile([128, 1], F32, tag="sh1")
        sc2_sb = const.tile([64, 1], F32, tag="sc2")
        sh2_sb = const.tile([64, 1], F32, tag="sh2")
        sc3_sb = const.tile([128, 4], F32, tag="sc3")
        sh3_sb = const.tile([128, 4], F32, tag="sh3")
        sc4_sb = const.tile([128, 2], F32, tag="sc4")
        sh4_sb = const.tile([128, 2], F32, tag="sh4")
        b3bc_sb = const.tile([128, 10], F32, tag="b3bc")
        sp_t = [const.tile([128, SP_FREE], BF16, tag=f"sp{i}", name=f"sp{i}")
                for i in range(NBUF)]
        ic2_t = [const.tile([96, IC2_FREE], BF16, tag=f"ic2_{i}", name=f"ic2_{i}")
                 for i in range(NBUF)]
        s2all = const.tile([64, 64 * BC], BF16, tag="s2all")   # (yx*256 + s)
        fc1in = const.tile([128, 32 * BC], BF16, tag="fc1in")  # (kt*256 + s)
        s3_sb = const.tile([128, 4, BC], BF16, tag="s3")
        u4_sb = const.tile([128, 2, BC], F32, tag="u4")
        s4h_sb = const.tile([128, 2, BC], BF16, tag="s4h")
        s4l_sb = const.tile([128, 2, BC], BF16, tag="s4l")
        s4r_sb = const.tile([128, 2, BC], F32, tag="s4r")

        # ---- setup DMAs ----
        nc.sync.dma_start(w1_sb[:], d_w1.ap())
        for dy in range(3):
            nc.sync.dma_start(w2_sb[:, dy, :], d_w2.ap()[dy])
        nc.sync.dma_start(fc1w_sb[:], bass.AP(
            tensor=d_fc1w, offset=0, ap=[[512, 128], [65536, 32], [1, 512]]))
        nc.sync.dma_start(fc2w_sb[:], bass.AP(
            tensor=d_fc2w, offset=0, ap=[[256, 128], [32768, 4], [1, 256]]))
        nc.sync.dma_start(w3h_sb[:], bass.AP(
            tensor=d_w3h, offset=0, ap=[[10, 128], [1280, 2], [1, 10]]))
        nc.sync.dma_start(w3l_sb[:], bass.AP(
            tensor=d_w3l, offset=0, ap=[[10, 128], [1280, 2], [1, 10]]))
        for t, d in [(sc1_sb, d_sc1), (sh1_sb, d_sh1), (sc2_sb, d_sc2),
                     (sh2_sb, d_sh2), (sc3_sb, d_sc3), (sh3_sb, d_sh3),
                     (sc4_sb, d_sc4), (sh4_sb, d_sh4), (b3bc_sb, d_b3bc)]:
            nc.sync.dma_start(t[:], d.ap())

        # zero the padded staging buffers once (pads stay zero forever)
        for t in sp_t:
            nc.vector.memset(t[:], 0.0)
        for t in ic2_t:
            nc.vector.memset(t[:], 0.0)

        def ap_of(t, dims, off=0):
            return bass.AP(tensor=t.tensor, offset=t.offset + off,
                           ap=[list(t.ap[0])] + [list(d) for d in dims])

        for _rep in range(reps):
            # ================= chunk loop =================
            for chk in range(NCHUNK):
                sp = sp_t[chk % NBUF]
                ic2 = ic2_t[chk % NBUF]

                ich = icp.tile([128, CH * 256], BF16, tag="ich")
                nc.sync.dma_start(ich[:], d_ic1h.ap()[chk])
                icl = icp.tile([128, CH * 256], BF16, tag="icl")
                nc.scalar.dma_start(icl[:], d_ic1l.ap()[chk])

                # ---- conv1: 4 psum tiles of 4 samples ----
                for t in range(4):
                    p1 = pp1.tile([128, 1024], F32, tag="c1")
                    for h in range(2):
                        sl = bass.ds(t * 1024 + h * 512, 512)
                        nc.tensor.matmul(p1[:, h * 512:(h + 1) * 512],
                                         w1_sb[:], ich[:, sl],
                                         start=True, stop=False)
                        nc.tensor.matmul(p1[:, h * 512:(h + 1) * 512],
                                         w1_sb[:], icl[:, sl],
                                         start=False, stop=True)
                    # DVE: 2x2 maxpool in ONE XY-window reduce from psum
                    pl1 = wk.tile([128, 256], F32, tag="pl1")
                    nc.vector.tensor_reduce(
                        ap_of(pl1, [[16, 16], [1, 16]]),
                        ap_of(p1, [[64, 16], [2, 16], [32, 2], [1, 2]]),
                        mybir.AxisListType.XY, ALU.max)
                    # ACT: sign(bn1) -> +-1 bf16 straight into sign1_pad
                    # [(co,j)128, (u: CH*18, s: 18, w: 1)]
                    nc.scalar.activation(
                        ap_of(sp, [[18, 4], [CH * 18, 4], [1, 16]],
                              (4 * t) * 18 + 1),
                        pl1[:], AF.Sign, bias=sh1_sb[:], scale=sc1_sb[:])

                # ---- ic2 build: one contiguous-run DMA per dx ----
                RUN = 4 * CH * 18
                for dx in range(3):
                    src = bass.AP(tensor=sp.tensor, offset=sp.offset + dx,
                                  ap=[list(sp.ap[0]), [1, RUN]])
                    dst_t = ic2[32 * dx:32 * (dx + 1)]
                    dst = bass.AP(tensor=dst_t.tensor,
                                  offset=dst_t.offset + CH * 18,
                                  ap=[list(dst_t.ap[0]), [RUN, 4], [1, RUN]])
                    eng = nc.sync if dx % 2 == 0 else nc.scalar
                    eng.dma_start(dst, src)

                # ---- conv2: 4 psum tiles of 4 samples ----
                for t in range(4):
                    p2 = pp2.tile([64, 1024], F32, tag="c2")
                    for h in range(2):
                        for dy in range(3):
                            mv = bass.AP(
                                tensor=ic2.tensor,
                                offset=ic2.offset + (4 * t + 2 * h) * 18
                                + dy * (CH * 18),
                                ap=[list(ic2.ap[0]), [18, 2], [CH * 18, 16],
                                    [1, 16]])
                            nc.tensor.matmul(p2[:, h * 512:(h + 1) * 512],
                                             w2_sb[:, dy, :], mv,
                                             start=(dy == 0), stop=(dy == 2))
                    # DVE: 2x2 maxpool in ONE XY-window reduce from psum
                    xm2b = wk.tile([64, 256], F32, tag="xm2b")
                    nc.vector.tensor_reduce(
                        ap_of(xm2b, [[8, 32], [1, 8]]),
                        ap_of(p2, [[32, 32], [2, 8], [16, 2], [1, 2]]),
                        mybir.AxisListType.XY, ALU.max)
                    # ACT sign(bn2) -> +-1 bf16 into s2all (yx*256 + s)
                    s0 = chk * CH + 4 * t
                    nc.scalar.activation(
                        ap_of(s2all, [[1, 4], [2048, 8], [256, 8]], s0),
                        xm2b[:], AF.Sign, bias=sh2_sb[:], scale=sc2_sb[:])

                # ---- two-wave repack s2all -> fc1in (overlaps chunks) ----
                if chk in (NCHUNK // 2 - 1, NCHUNK - 1):
                    half = 0 if chk == NCHUNK // 2 - 1 else 1
                    HB = BC // 2
                    for kt in range(32):
                        for c2 in range(2):
                            src_t = s2all[2 * kt + c2:2 * kt + c2 + 1]
                            src = bass.AP(tensor=src_t.tensor,
                                          offset=src_t.offset + half * HB,
                                          ap=[list(src_t.ap[0]), [256, 64],
                                              [1, HB]])
                            dst_t = fc1in[64 * c2:64 * (c2 + 1)]
                            dst = bass.AP(tensor=dst_t.tensor,
                                          offset=dst_t.offset + kt * BC
                                          + half * HB,
                                          ap=[list(dst_t.ap[0]), [1, HB]])
                            eng = nc.sync if (kt + c2) % 2 == 0 else nc.scalar
                            eng.dma_start(dst, src)

            # ================= fc phase =================
            # fc1: 4 m-tiles
            for m in range(4):
                pf = pp1.tile([128, 1024], F32, tag="c1")
                for kt in range(32):
                    nc.tensor.matmul(pf[:, :BC],
                                     fc1w_sb[:, kt, 128 * m:128 * (m + 1)],
                                     fc1in[:, kt * BC:(kt + 1) * BC],
                                     start=(kt == 0), stop=(kt == 31))
                nc.scalar.activation(s3_sb[:, m, :], pf[:, :BC], AF.Sign,
                                     bias=sh3_sb[:, m:m + 1],
                                     scale=sc3_sb[:, m:m + 1])

            # fc2: 2 m-tiles
            for m2 in range(2):
                pg = pp1.tile([128, 1024], F32, tag="c1")
                for kt in range(4):
                    nc.tensor.matmul(pg[:, :BC],
                                     fc2w_sb[:, kt, 128 * m2:128 * (m2 + 1)],
                                     s3_sb[:, kt, :],
                                     start=(kt == 0), stop=(kt == 3))
                nc.scalar.activation(u4_sb[:, m2, :], pg[:, :BC], AF.Identity,
                                     bias=sh4_sb[:, m2:m2 + 1],
                                     scale=sc4_sb[:, m2:m2 + 1])

            # clip to [-1,1] in one dual-op DVE instruction
            nc.vector.tensor_scalar(u4_sb[:], u4_sb[:], 1.0, -1.0,
                                    ALU.min, ALU.max)
            # hi/lo split of s4
            nc.scalar.copy(s4h_sb[:], u4_sb[:])                   # bf16 round
            nc.vector.tensor_sub(s4r_sb[:], u4_sb[:], s4h_sb[:])  # residual
            nc.scalar.copy(s4l_sb[:], s4r_sb[:])                  # bf16 resid

            # fc3 + log_softmax, 2 batch tiles of 128
            for bt in range(2):
                ph = pp1.tile([128, 1024], F32, tag="c1")
                mms = []
                for kt in range(2):
                    lh = s4h_sb[:, kt, 128 * bt:128 * (bt + 1)]
                    ll = s4l_sb[:, kt, 128 * bt:128 * (bt + 1)]
                    mms += [(lh, w3h_sb[:, kt, :]), (ll, w3h_sb[:, kt, :]),
                            (lh, w3l_sb[:, kt, :])]
                for i, (lhs, rhs) in enumerate(mms):
                    nc.tensor.matmul(ph[:, :10], lhs, rhs,
                                     start=(i == 0), stop=(i == len(mms) - 1))
                h3 = wk.tile([128, 10], F32, tag="h3")
                nc.vector.tensor_add(h3[:], ph[:, :10], b3bc_sb[:])
                mx = wk.tile([128, 1], F32, tag="mx")
                nc.vector.tensor_reduce(mx[:], h3[:], mybir.AxisListType.X,
                                        ALU.max)
                negmx = wk.tile([128, 1], F32, tag="negmx")
                nc.scalar.mul(negmx[:], mx[:], -1.0)
                et = wk.tile([128, 10], F32, tag="et")
                se = wk.tile([128, 1], F32, tag="se")
                nc.scalar.activation(et[:], h3[:], AF.Exp, bias=negmx[:],
                                     scale=1.0, accum_out=se[:])
                ls = wk.tile([128, 1], F32, tag="ls")
                nc.scalar.activation(ls[:], se[:], AF.Ln)
                tt = wk.tile([128, 1], F32, tag="tt")
                nc.vector.tensor_add(tt[:], mx[:], ls[:])
                negt = wk.tile([128, 1], F32, tag="negt")
                nc.scalar.mul(negt[:], tt[:], -1.0)
                o = wk.tile([128, 10], F32, tag="o")
                nc.scalar.activation(o[:], h3[:], AF.Identity, bias=negt[:],
                                     scale=1.0)
                nc.sync.dma_start(d_out.ap()[128 * bt:128 * (bt + 1), :], o[:])

    nc.compile()
    return nc


def _get_module():
    if "nc" not in _nc_cache:
        _nc_cache["nc"] = _build_module()
    return _nc_cache["nc"]


def kernel(**inputs):
    from concourse.bass_utils import run_bass_kernel_spmd

    in_maps = _host_prep(inputs)
    nc = _get_module()
    res = run_bass_kernel_spmd(nc, in_maps, core_ids=list(range(NCORES)))
    out = np.concatenate([r["out"] for r in res.results], axis=0)
    return out.astype(np.float32)

